# revision 48
# baseline (speedup 1.0000x reference)
"""Trainium2 Bass kernel for nn_EqAMPBC (FWM/XPM nonlinear equalizer), v2.

Data-parallel over 8 cores (batch 131072 -> 16384/core), 32 chunks of
N=512 samples in transposed layout (features on partitions, samples free).

v2 structure (vs v1):
  - Conjugate pairing: S[-m, m+n] = conj(S[m, n]) halves the triplet set
    to R=187 representatives; the conjugate contribution is folded into
    the w2 weight tables.
  - Gathers run as fp8e4 DoubleRow matmuls (half PE cost); the rhs is an
    interleaved [82, 2, N] (re, im) fp8 tile so one gather matmul can pull
    arbitrary mixes of real/imag rows.
  - Products are free-dim packed: one DVE op computes (ar*cr | ai*ci).
  - S crosses PSUM->SBUF via DMA (f32) and w2 runs with float32r moving
    operands (1 cycle/row), freeing Act/Pool for the gather copies.
  - Phase/center path (q, squares, xrA/B, r3, final) keeps v1 semantics.
"""
import sys
import numpy as np

sys.path.insert(0, "/opt/trn_rl_repo")

M = 41
P = 20
RHO = 1.0
NCORES = 8
N = 512
F_ROWS = 8
SLAB_REPS = 64  # reps per slab (2 product rows per rep)
R_KEEP = 187    # kept representatives; 187 = exact (lossy drop is too lossy)


def _fwm_index():
    h = M // 2
    ms, ns = [], []
    for m in range(-h, h + 1):
        for n in range(-h, h + 1):
            if m != 0 and n != 0 and abs(m * n) <= RHO * h and abs(m + n) <= h:
                ms.append(m)
                ns.append(n)
    return np.asarray(ms, np.int32), np.asarray(ns, np.int32)


M_IDX, N_IDX = _fwm_index()
HDIM = len(M_IDX)
H_OF = {(int(M_IDX[i]), int(N_IDX[i])): i for i in range(HDIM)}


def _build_reps():
    """Representatives under (m, n) -> (-m, m+n) conjugate pairing."""
    reps = []
    seen = set()
    for i in range(HDIM):
        m, n = int(M_IDX[i]), int(N_IDX[i])
        if (m, n) in seen:
            continue
        pm, pn = -m, m + n
        if (pm, pn) in H_OF and (pm, pn) != (m, n):
            reps.append((m, n, pm, pn))
            seen.add((m, n))
            seen.add((pm, pn))
        else:
            reps.append((m, n, None, None))
            seen.add((m, n))
    return reps


ALL_REPS = _build_reps()  # 187 representatives
R = min(R_KEEP, len(ALL_REPS))
SLABS = [min(SLAB_REPS, R - o) for o in range(0, R, SLAB_REPS)]
NSLAB = len(SLABS)
SLAB_OFF = [sum(SLABS[:i]) for i in range(NSLAB)]


def _select_reps(fwm_wr, fwm_wi):
    """Top-R representatives by conjugate-pair weight mass."""
    mass = []
    for (m, n, pm, pn) in ALL_REPS:
        h = H_OF[(m, n)]
        v = float(np.sum(fwm_wr[:, h] ** 2 + fwm_wi[:, h] ** 2))
        if pm is not None:
            hp = H_OF[(pm, pn)]
            v += float(np.sum(fwm_wr[:, hp] ** 2 + fwm_wi[:, hp] ** 2))
        mass.append(v)
    order = np.argsort(mass)[::-1][:R]
    return [ALL_REPS[i] for i in sorted(order)]


def _build_tables(fwm_wr, fwm_wi, conv1_w, conv2_w, C00):
    """Build packed constant tables.

    Returns dict:
      CT8  [128, sum(2*rows_s)*4] fp8-valued f32 (caller converts): DR gather
           one-hots, order: slab-major, group (ar, ai, cr, ci), layout per
           block = (comp-major, rows) flattened.
      CTB  [128, ...] bf16: w1 tables (re, imp, imm per slab) then r3 tables.
      CTF  [128, ...] f32: w2 (re, im per slab).
      plus r3 offsets metadata handled by fixed layout below.
    """
    t = {}
    reps = _select_reps(fwm_wr, fwm_wi)
    # --- gather tables (bf16, one-hot; rhs is x[:, comp, :]) ---
    blocks8 = []
    for s in range(NSLAB):
        rows = 2 * SLABS[s]
        for g in range(4):  # ar, ai, cr, ci
            tab = np.zeros((128, rows), np.float32)
            for rl in range(SLABS[s]):
                m, n, _, _ = reps[SLAB_OFF[s] + rl]
                tap = P + n if g < 2 else P + m + n
                for j in range(2):
                    tab[j * 41 + tap, 2 * rl + j] = 1.0
            blocks8.append(tab)
    t["CT8"] = np.concatenate(blocks8, axis=1)

    # --- w1 tables (bf16, +-1) ---
    blocksb = []
    for s in range(NSLAB):
        rows = 2 * SLABS[s]
        t_re = np.zeros((128, rows), np.float32)
        t_ip = np.zeros((128, rows), np.float32)
        t_im = np.zeros((128, rows), np.float32)
        for rl in range(SLABS[s]):
            for j in range(2):
                t_re[2 * rl + j, 2 * rl] = 1.0
                t_ip[2 * rl + j, 2 * rl + 1] = 1.0
                t_im[2 * rl + j, 2 * rl + 1] = -1.0
        blocksb += [t_re, t_ip, t_im]
    # --- r3 tables (bf16) ---
    w1z = conv1_w.copy(); w1z[P] = 0.0
    w2z = conv2_w.copy(); w2z[P] = 0.0
    q1 = np.zeros((128, F_ROWS), np.float32)
    q2 = np.zeros((128, F_ROWS), np.float32)
    q3 = np.zeros((128, F_ROWS), np.float32)
    q4 = np.zeros((128, F_ROWS), np.float32)
    pw = np.zeros((128, F_ROWS), np.float32)
    for i in range(2):
        rows = np.arange(41) + i * 41
        q1[rows, 2 * i] = 0.5
        q2[rows, 2 * i] = -0.5
        q3[rows, 2 * i + 1] = 0.5
        q4[rows, 2 * i + 1] = 0.5
        for tap in range(41):
            r = i * 41 + tap
            pw[r, 6] += (2.0 if i == 0 else 1.0) * w1z[tap]
            pw[r, 7] += (2.0 if i == 1 else 1.0) * w1z[tap]
        pw[i * 41 + P, 6] += 0.5 * C00
        pw[i * 41 + P, 7] += 0.5 * C00
    xrA = np.zeros((128, F_ROWS), np.float32)
    xrA[np.arange(41), 4] = 0.5 * w2z
    xrA[np.arange(41) + 64, 4] = 0.5 * w2z
    xrB = np.zeros((128, F_ROWS), np.float32)
    xrB[np.arange(41), 5] = 0.5 * w2z
    xrB[np.arange(41) + 64, 5] = -0.5 * w2z
    blocksb += [q1, q2, q3, q4, pw, xrA, xrB]
    t["CTB"] = np.concatenate(blocksb, axis=1)

    # --- w2 tables (f32, conj fold) ---
    blocksf = []
    for s in range(NSLAB):
        rows = 2 * SLABS[s]
        wre = np.zeros((128, 82), np.float32)
        wim = np.zeros((128, 82), np.float32)
        for rl in range(SLABS[s]):
            m, n, pm, pn = reps[SLAB_OFF[s] + rl]
            h = H_OF[(m, n)]
            for i in range(2):
                col = i * 41 + P + m
                wre[2 * rl + 0, col] += fwm_wr[i, h]
                wre[2 * rl + 1, col] += -fwm_wi[i, h]
                wim[2 * rl + 0, col] += fwm_wi[i, h]
                wim[2 * rl + 1, col] += fwm_wr[i, h]
            if pm is not None:
                hp = H_OF[(pm, pn)]
                for i in range(2):
                    col = i * 41 + P + pm
                    wre[2 * rl + 0, col] += fwm_wr[i, hp]
                    wre[2 * rl + 1, col] += fwm_wi[i, hp]
                    wim[2 * rl + 0, col] += fwm_wi[i, hp]
                    wim[2 * rl + 1, col] += -fwm_wr[i, hp]
        blocksf += [wre, wim]
    t["CTF"] = np.concatenate(blocksf, axis=1)  # bf16 on device
    t["ident8"] = np.eye(F_ROWS, dtype=np.float32)
    return t


_CACHED = {}


def _build_program(Bc):
    import concourse.bacc as bacc
    import concourse.mybir as mybir
    import concourse.tile as tile

    f32 = mybir.dt.float32
    bf16 = mybir.dt.bfloat16
    Act = mybir.ActivationFunctionType
    Op = mybir.AluOpType
    NCHUNK = Bc // N
    MCOLS = Bc // 128

    # const layout offsets
    ct8_off, o = [], 0
    for s in range(NSLAB):
        for g in range(4):
            ct8_off.append(o)
            o += 2 * SLABS[s]
    CT8_COLS = o
    ctb_off, o = {}, 0
    for s in range(NSLAB):
        for nm in ("re", "ip", "im"):
            ctb_off[(s, nm)] = o
            o += 2 * SLABS[s]
    for nm in ("q1", "q2", "q3", "q4", "pw", "xrA", "xrB"):
        ctb_off[nm] = o
        o += F_ROWS
    CTB_COLS = o
    ctf_off, o = {}, 0
    for s in range(NSLAB):
        for nm in ("re", "im"):
            ctf_off[(s, nm)] = o
            o += 82
    CTF_COLS = o

    nc = bacc.Bacc("TRN2", target_bir_lowering=False, debug=False,
                   num_devices=NCORES)

    dXP = nc.dram_tensor("XP", [82, 2, Bc], bf16, kind="ExternalInput").ap()
    dXP2 = nc.dram_tensor("XP2", [82, 2, Bc], bf16, kind="ExternalInput").ap()
    dXC = nc.dram_tensor("XC", [128, 4 * MCOLS], f32, kind="ExternalInput").ap()
    dT0 = nc.dram_tensor("T0M", [128, MCOLS], f32, kind="ExternalInput").ap()
    dCT8 = nc.dram_tensor("CT8", [128, CT8_COLS], bf16, kind="ExternalInput").ap()
    dCTB = nc.dram_tensor("CTB", [128, CTB_COLS], bf16, kind="ExternalInput").ap()
    dCTF = nc.dram_tensor("CTF", [128, CTF_COLS], bf16, kind="ExternalInput").ap()
    dID8 = nc.dram_tensor("ID8F", [F_ROWS, F_ROWS], f32,
                          kind="ExternalInput").ap()
    dOUT = nc.dram_tensor("OUT", [128, 4 * MCOLS], f32,
                          kind="ExternalOutput").ap()

    with tile.TileContext(nc) as tc:
        with (
            tc.tile_pool(name="consts", bufs=1) as cpool,
            tc.tile_pool(name="xin", bufs=3) as xpool,
            tc.tile_pool(name="gsb", bufs=3) as gpool,
            tc.tile_pool(name="prod", bufs=2) as ppool,
            tc.tile_pool(name="ssb", bufs=2) as spool,
            tc.tile_pool(name="mid", bufs=2) as mpool,
            tc.tile_pool(name="persist", bufs=1) as zpool,
            tc.tile_pool(name="fin", bufs=4) as fpool,
            tc.tile_pool(name="pga", bufs=1, space="PSUM") as pga,
            tc.tile_pool(name="pgc", bufs=1, space="PSUM") as pgc,
            tc.tile_pool(name="pss", bufs=1, space="PSUM") as pss,
            tc.tile_pool(name="pvv", bufs=1, space="PSUM") as pvv,
            tc.tile_pool(name="pff", bufs=1, space="PSUM") as pff,
            tc.tile_pool(name="pgg", bufs=1, space="PSUM") as pgg,
        ):
            # ---- constants (gather tables first; x DMAs slot in between) ----
            ct8 = cpool.tile([128, CT8_COLS], bf16, tag="ct8", name="ct8")
            ctb = cpool.tile([128, CTB_COLS], bf16, tag="ctb", name="ctb")
            ctf = cpool.tile([128, CTF_COLS], bf16, tag="ctf", name="ctf")
            t0m = cpool.tile([128, MCOLS], f32, tag="t0m", name="t0m")
            xcs = cpool.tile([128, 4 * MCOLS], f32, tag="xcs", name="xcs")
            ident8f = cpool.tile([F_ROWS, F_ROWS], f32, tag="id8",
                                 name="ident8f")
            CT8_S0 = ct8_off[4]  # end of slab-0 tables
            nc.sync.dma_start(ct8[:, 0:CT8_S0], dCT8[:, 0:CT8_S0])

            def gq(s, g):
                rows = 2 * SLABS[s]
                off = ct8_off[4 * s + g]
                return ct8[0:82, off:off + rows]

            def w1t(s, nm):
                rows = 2 * SLABS[s]
                off = ctb_off[(s, nm)]
                return ctb[0:rows, off:off + rows]

            def w2t(s, nm):
                rows = 2 * SLABS[s]
                off = ctf_off[(s, nm)]
                return ctf[0:rows, off:off + 82]

            def r3t(nm, np_=128):
                off = ctb_off[nm]
                return ctb[0:np_, off:off + F_ROWS]

            Mt = zpool.tile([128, NCHUNK * 32], f32, tag="mega", name="mega")
            xrAB = []
            for par in range(2):
                a = zpool.tile([128, N], bf16, tag=f"xrA{par}", name=f"xrA{par}")
                b = zpool.tile([128, N], bf16, tag=f"xrB{par}", name=f"xrB{par}")
                nc.vector.memset(a[:], 0.0)
                nc.vector.memset(b[:], 0.0)
                xrAB.append((a, b))
            OUTs = zpool.tile([128, 4 * MCOLS], f32, tag="outs", name="outs")

            # ---- software-pipelined chunk loop ----
            def new_state(c):
                st = {"c": c}
                st["x"] = xpool.tile([82, 2, N], bf16, tag="x", name="x")
                st["x2"] = xpool.tile([82, 2, N], bf16, tag="x2", name="x2")
                st["xm1"] = xpool.tile([41, 2, N], bf16, tag="xm1", name="xm1")
                st["xrA"], st["xrB"] = xrAB[c % 2]
                return st

            def emit_dmas(st):
                cs = slice(st["c"] * N, (st["c"] + 1) * N)
                nc.sync.dma_start(st["x"][:], dXP[:, :, cs])
                nc.sync.dma_start(st["x2"][:], dXP2[:, :, cs])
                nc.sync.dma_start(st["xm1"][:], dXP[41:82, :, cs])

            def emit_gather_a(st, s):
                rows = 2 * SLABS[s]
                pGA = pga.tile([128, 2, N], f32, tag="pga", name="pGA")
                for g in (0, 1):
                    nc.tensor.matmul(pGA[0:rows, g, :], gq(s, g),
                                     st["x"][:, g % 2, :],
                                     start=True, stop=True)
                st[f"pGA{s}"] = pGA

            def emit_gather_c(st, s):
                rows = 2 * SLABS[s]
                pGC = pgc.tile([128, 2, N], f32, tag="pgc", name="pGC")
                for g in (2, 3):
                    nc.tensor.matmul(pGC[0:rows, g - 2, :], gq(s, g),
                                     st["x"][:, g % 2, :],
                                     start=True, stop=True)
                st[f"pGC{s}"] = pGC

            def emit_ga_copy(st, s):
                rows = 2 * SLABS[s]
                ga = gpool.tile([128, 2, N], bf16, tag="gas", name="ga")
                nc.scalar.activation(ga[0:rows, :, :],
                                     st[f"pGA{s}"][0:rows, :, :], Act.Copy)
                st[f"ga{s}"] = ga

            def emit_gc_copy(st, s):
                rows = 2 * SLABS[s]
                gc = gpool.tile([128, 2, N], bf16, tag="gcs", name="gc")
                nc.scalar.activation(gc[0:rows, :, :],
                                     st[f"pGC{s}"][0:rows, :, :], Act.Copy)
                st[f"gc{s}"] = gc

            def emit_products(st, s, eng):
                rows = 2 * SLABS[s]
                ga, gc = st[f"ga{s}"], st[f"gc{s}"]
                p12 = ppool.tile([128, 2, N], bf16, tag="p12", name="p12")
                p3 = ppool.tile([128, N], bf16, tag="p3", name="p3")
                p4 = ppool.tile([128, N], bf16, tag="p4", name="p4")
                nc.vector.tensor_tensor(p12[0:rows, :, :], ga[0:rows, :, :],
                                        gc[0:rows, :, :], Op.mult)
                nc_e = nc.gpsimd if eng == "pool" else nc.vector
                nc_e.tensor_tensor(p3[0:rows, :], ga[0:rows, 1, :],
                                   gc[0:rows, 0, :], Op.mult)
                nc_e.tensor_tensor(p4[0:rows, :], ga[0:rows, 0, :],
                                   gc[0:rows, 1, :], Op.mult)
                st[f"p12{s}"], st[f"p3{s}"], st[f"p4{s}"] = p12, p3, p4

            def emit_w1(st, s):
                rows = 2 * SLABS[s]
                pS = pss.tile([128, N], f32, tag="s", name="pS")
                nc.tensor.matmul(pS[0:rows, :], w1t(s, "re"),
                                 st[f"p12{s}"][0:rows, 0, :],
                                 start=True, stop=False)
                nc.tensor.matmul(pS[0:rows, :], w1t(s, "re"),
                                 st[f"p12{s}"][0:rows, 1, :],
                                 start=False, stop=False)
                nc.tensor.matmul(pS[0:rows, :], w1t(s, "ip"),
                                 st[f"p3{s}"][0:rows, :],
                                 start=False, stop=False)
                nc.tensor.matmul(pS[0:rows, :], w1t(s, "im"),
                                 st[f"p4{s}"][0:rows, :],
                                 start=False, stop=True)
                st[f"pS{s}"] = pS

            def emit_scopy(st, s):
                rows = 2 * SLABS[s]
                ss = spool.tile([128, N], bf16, tag=f"ss{s}", name=f"ss{s}")
                nc.vector.tensor_copy(ss[0:rows, :], st[f"pS{s}"][0:rows, :])
                st[f"ss{s}"] = ss

            def emit_w2_mm(st, nm):
                if "pV" not in st:
                    st["pV"] = pvv.tile([82, N], f32, tag="v", name="pV")
                    st["VS"] = mpool.tile([82, 2, N], bf16, tag="vs", name="VS")
                for s in range(NSLAB):
                    rows = 2 * SLABS[s]
                    nc.tensor.matmul(st["pV"][:], w2t(s, nm),
                                     st[f"ss{s}"][0:rows, :],
                                     start=(s == 0), stop=(s == NSLAB - 1))

            def emit_v_copy(st, ci):
                if ci == 0:
                    nc.scalar.activation(st["VS"][:, 0, :], st["pV"][:],
                                         Act.Copy)
                else:
                    nc.vector.tensor_copy(st["VS"][:, 1, :], st["pV"][:])

            def emit_q(st):
                st["Q12"] = mpool.tile([82, 2, N], bf16, tag="q12", name="Q12")
                st["Q34"] = mpool.tile([82, 2, N], bf16, tag="q34", name="Q34")
                nc.vector.tensor_tensor(st["Q12"][:], st["VS"][:], st["x"][:],
                                        Op.mult)
                nc.vector.tensor_tensor(st["Q34"][:], st["VS"][:], st["x2"][:],
                                        Op.mult)

            def emit_s12_xr(st):
                x, xm1 = st["x"], st["xm1"]
                st["S12"] = mpool.tile([82, 2, N], bf16, tag="s12", name="S12")
                nc.vector.tensor_tensor(st["S12"][:], x[:], x[:], Op.mult)
                nc.vector.tensor_tensor(st["xrA"][0:41, :], x[0:41, 0, :],
                                        xm1[:, 0, :], Op.mult)
                nc.gpsimd.tensor_tensor(st["xrA"][64:105, :], x[0:41, 1, :],
                                        xm1[:, 1, :], Op.mult)
                nc.vector.tensor_tensor(st["xrB"][0:41, :], x[0:41, 1, :],
                                        xm1[:, 0, :], Op.mult)
                nc.gpsimd.tensor_tensor(st["xrB"][64:105, :], x[0:41, 0, :],
                                        xm1[:, 1, :], Op.mult)

            def emit_r3(st):
                pF = pff.tile([F_ROWS, N], f32, tag="f", name="pF")
                seq = [
                    (r3t("q1", 82), st["Q12"][:, 0, :]),
                    (r3t("q2", 82), st["Q12"][:, 1, :]),
                    (r3t("q3", 82), st["Q34"][:, 0, :]),
                    (r3t("q4", 82), st["Q34"][:, 1, :]),
                    (r3t("pw", 82), st["S12"][:, 0, :]),
                    (r3t("pw", 82), st["S12"][:, 1, :]),
                    (r3t("xrA", 128), st["xrA"][:]),
                    (r3t("xrB", 128), st["xrB"][:]),
                ]
                for si, (wt, rhs) in enumerate(seq):
                    nc.tensor.matmul(pF[:], wt, rhs,
                                     start=(si == 0), stop=(si == len(seq) - 1))
                st["pF"] = pF

            def emit_sf_tr_mt(st):
                c = st["c"]
                sF = mpool.tile([F_ROWS, N], f32, tag="sF", name="sF")
                nc.scalar.activation(sF[:], st["pF"][:], Act.Copy)
                pG = pgg.tile([128, 32], f32, tag="g", name="pG")
                for tq in range(4):
                    nc.tensor.transpose(pG[:, tq * 8:tq * 8 + 8],
                                        sF[:, tq * 128:(tq + 1) * 128],
                                        ident8f[:])
                nc.vector.tensor_copy(Mt[:, c * 32:(c + 1) * 32], pG[:])

            # ---- final sample-major phase, emitted in column halves ----
            Mtv = Mt[:].rearrange("p (g k) -> p g k", k=8)
            hpi = cpool.tile([128, 1], f32, tag="hpi", name="hpi")
            LN10_10 = float(np.log(10.0) / 10.0)

            FIN_COMBOS = [
                (0, [(2, 5, -1.0), (3, 4, -1.0)], (0, "c0", +1.0), (1, "s0", -1.0), 0),
                (1, [(2, 4, +1.0), (3, 5, -1.0)], (0, "s0", +1.0), (1, "c0", +1.0), 1),
                (2, [(0, 5, +1.0), (1, 4, -1.0)], (2, "c1", +1.0), (3, "s1", -1.0), 2),
                (3, [(0, 4, +1.0), (1, 5, +1.0)], (2, "s1", +1.0), (3, "c1", +1.0), 3),
            ]

            def emit_final_trig(fs, g0, g1, tg):
                gw = g1 - g0

                def ft(tag):
                    return fpool.tile([128, gw], f32, tag=tag + tg,
                                      name="ftmp")

                fs["Pht"] = ft("fA")
                nc.scalar.activation(fs["Pht"][:], t0m[:, g0:g1], Act.Exp,
                                     scale=LN10_10)
                phi0, phi1 = ft("fB0"), ft("fB1")
                ve = fs["ve"]
                ve.tensor_tensor(phi0[:], fs["Pht"][:], Mtv[:, g0:g1, 6],
                                 Op.mult)
                ve.tensor_tensor(phi1[:], fs["Pht"][:], Mtv[:, g0:g1, 7],
                                 Op.mult)
                tr = {}
                for nm, ph, bias in (("c0", phi0, True), ("s0", phi0, False),
                                     ("c1", phi1, True), ("s1", phi1, False)):
                    t = ft("fC" + nm)
                    if bias:
                        nc.scalar.activation(t[:], ph[:], Act.Sin, bias=hpi[:])
                    else:
                        nc.scalar.activation(t[:], ph[:], Act.Sin)
                    tr[nm] = t
                fs["trig"] = tr

            def emit_final_combo(fs, g0, g1, combo, tg):
                gw = g1 - g0
                fidx, prods, term1, term2, outq = combo
                ve = fs["ve"]

                def xcb(q):
                    return xcs[:, q * MCOLS + g0:q * MCOLS + g1]

                acc = fpool.tile([128, gw], f32, tag="fD" + tg, name="facc")
                ve.tensor_copy(acc[:], Mtv[:, g0:g1, fidx])
                for (ka, kb, sgn) in prods:
                    tmp = fpool.tile([128, gw], f32, tag="fE" + tg, name="ftp")
                    ve.tensor_tensor(tmp[:], xcb(ka), Mtv[:, g0:g1, kb],
                                     Op.mult)
                    ve.tensor_tensor(acc[:], acc[:], tmp[:],
                                     Op.add if sgn > 0 else Op.subtract)
                ve.tensor_tensor(acc[:], acc[:], fs["Pht"][:], Op.mult)
                for (kc, tkey, sgn) in (term1, term2):
                    tmp = fpool.tile([128, gw], f32, tag="fE" + tg, name="ftp")
                    ve.tensor_tensor(tmp[:], xcb(kc), fs["trig"][tkey][:],
                                     Op.mult)
                    ve.tensor_tensor(acc[:], acc[:], tmp[:],
                                     Op.add if sgn > 0 else Op.subtract)
                ve.tensor_copy(OUTs[:, outq * MCOLS + g0:outq * MCOLS + g1],
                               acc[:])

            # ---- pipelined main loop ----
            nxt = new_state(0)
            emit_dmas(nxt)
            nc.sync.dma_start(ct8[:, CT8_S0:], dCT8[:, CT8_S0:])
            nc.sync.dma_start(ctb[:], dCTB[:])
            nc.sync.dma_start(ctf[:], dCTF[:])
            nc.sync.dma_start(t0m[:], dT0[:])
            nc.sync.dma_start(xcs[:], dXC[:])
            nc.sync.dma_start(ident8f[:], dID8[:])
            nc.vector.memset(hpi[:], float(np.pi / 2))
            prv = None
            for i in range(NCHUNK + 1):
                cur = nxt if i < NCHUNK else None
                nxt = new_state(i + 1) if i + 1 < NCHUNK else None
                if nxt is not None:
                    emit_dmas(nxt)
                if cur is not None:
                    emit_gather_a(cur, 0)       # PE x2
                if prv is not None:
                    emit_w2_mm(prv, "re")       # PE x3
                    emit_v_copy(prv, 0)         # Act Vre
                if cur is not None:
                    emit_ga_copy(cur, 0)        # Act
                    emit_gather_c(cur, 0)       # PE x2
                if prv is not None:
                    emit_w2_mm(prv, "im")       # PE x3
                    emit_v_copy(prv, 1)         # DVE Vim
                if cur is not None:
                    emit_gc_copy(cur, 0)        # Act
                if prv is not None:
                    emit_q(prv)                 # DVE x2
                if cur is not None:
                    emit_gather_a(cur, 1)       # PE x2
                    emit_products(cur, 0, "pool")
                    emit_ga_copy(cur, 1)        # Act
                if prv is not None:
                    emit_r3(prv)                # PE x8
                if cur is not None:
                    emit_gather_c(cur, 1)       # PE x2
                    emit_gc_copy(cur, 1)        # Act
                    emit_w1(cur, 0)             # PE x4
                    emit_scopy(cur, 0)          # DVE
                    emit_gather_a(cur, 2)       # PE x2
                    emit_products(cur, 1, "pool")
                    emit_ga_copy(cur, 2)        # Act
                if prv is not None:
                    emit_sf_tr_mt(prv)          # Act sF + PE tr + DVE Mt
                if cur is not None:
                    emit_gather_c(cur, 2)       # PE x2
                    emit_gc_copy(cur, 2)        # Act
                    emit_w1(cur, 1)             # PE x4
                    emit_scopy(cur, 1)          # DVE
                    emit_products(cur, 2, "dve")
                    emit_s12_xr(cur)            # DVE + Pool
                    emit_w1(cur, 2)             # PE x4
                    emit_scopy(cur, 2)          # DVE
                # first-half final phase trickled on Pool after chunk 15 done
                HM = MCOLS // 2
                if i == NCHUNK // 2:
                    finA = {"ve": nc.gpsimd}
                    emit_final_trig(finA, 0, HM, "A")
                elif NCHUNK // 2 < i < NCHUNK // 2 + 5:
                    emit_final_combo(finA, 0, HM,
                                     FIN_COMBOS[i - NCHUNK // 2 - 1], "A")
                elif i == NCHUNK // 2 + 5:
                    nc.sync.dma_start(
                        dOUT[:].rearrange("p (q m) -> p q m", q=4)[:, :, 0:HM],
                        OUTs[:].rearrange("p (q m) -> p q m", q=4)[:, :, 0:HM])
                prv = cur
            # second half at the end, split across DVE and Pool
            finB = {"ve": nc.vector}
            finC = {"ve": nc.gpsimd}
            emit_final_trig(finB, HM, MCOLS, "B")
            finC["Pht"], finC["trig"] = finB["Pht"], finB["trig"]
            emit_final_combo(finB, HM, MCOLS, FIN_COMBOS[0], "B")
            emit_final_combo(finC, HM, MCOLS, FIN_COMBOS[1], "C")
            emit_final_combo(finB, HM, MCOLS, FIN_COMBOS[2], "B")
            emit_final_combo(finC, HM, MCOLS, FIN_COMBOS[3], "C")
            nc.sync.dma_start(
                dOUT[:].rearrange("p (q m) -> p q m", q=4)[:, :, HM:MCOLS],
                OUTs[:].rearrange("p (q m) -> p q m", q=4)[:, :, HM:MCOLS])

    nc.compile()
    return nc


def kernel(**inputs):
    from concourse.bass_utils import run_bass_kernel_spmd
    import ml_dtypes

    trace = bool(inputs.pop("_trace", False))
    x_real = np.asarray(inputs["x_real"], dtype=np.float32)
    x_imag = np.asarray(inputs["x_imag"], dtype=np.float32)
    task_info = np.asarray(inputs["task_info"], dtype=np.float32)
    C00 = float(np.asarray(inputs["C00"]).reshape(-1)[0])
    fwm_wr = np.asarray(inputs["fwm_wr"], dtype=np.float32)
    fwm_wi = np.asarray(inputs["fwm_wi"], dtype=np.float32)
    conv1_w = np.asarray(inputs["conv1_w"], dtype=np.float32)
    conv2_w = np.asarray(inputs["conv2_w"], dtype=np.float32)

    B = x_real.shape[0]
    Bc = B // NCORES
    if "nc" not in _CACHED:
        _CACHED["nc"] = _build_program(Bc)
    nc = _CACHED["nc"]

    bf = ml_dtypes.bfloat16
    tabs = _build_tables(fwm_wr, fwm_wi, conv1_w, conv2_w, C00)
    CT8 = tabs["CT8"].astype(bf)
    CTB = tabs["CTB"].astype(bf)
    CTF = tabs["CTF"].astype(bf)
    ID8 = tabs["ident8"]

    in_maps = []
    for core in range(NCORES):
        sl = slice(core * Bc, (core + 1) * Bc)
        XPr = np.ascontiguousarray(
            x_real[sl].transpose(2, 1, 0).reshape(82, Bc))
        XPi = np.ascontiguousarray(
            x_imag[sl].transpose(2, 1, 0).reshape(82, Bc))
        XP = np.ascontiguousarray(np.stack([XPr, XPi], axis=1)).astype(bf)
        XP2 = np.ascontiguousarray(np.stack([XPi, XPr], axis=1)).astype(bf)
        t0 = task_info[sl, 0]
        T0M = np.ascontiguousarray(
            t0.reshape(Bc // 512, 4, 128).transpose(2, 0, 1).reshape(128, Bc // 128))
        mcols = Bc // 128
        XC = np.empty((128, 4 * mcols), np.float32)
        for qi, arr in enumerate([x_real[sl, P, 0], x_imag[sl, P, 0],
                                  x_real[sl, P, 1], x_imag[sl, P, 1]]):
            XC[:, qi * mcols:(qi + 1) * mcols] = np.ascontiguousarray(
                arr.reshape(Bc // 512, 4, 128).transpose(2, 0, 1).reshape(128, mcols))
        m = {"XP": XP, "XP2": XP2, "T0M": T0M, "XC": XC,
             "CT8": CT8, "CTB": CTB, "CTF": CTF, "ID8F": ID8}
        in_maps.append(m)

    res = run_bass_kernel_spmd(nc, in_maps, list(range(NCORES)), trace=trace)
    _CACHED["last_exec_ns"] = res.exec_time_ns

    outs = []
    cols = Bc // 128
    for core in range(NCORES):
        OUT = res.results[core]["OUT"]
        E = np.empty((Bc, 2), np.complex64)
        for q, (dst, im) in enumerate([(0, 0), (0, 1), (1, 0), (1, 1)]):
            O = OUT[:, q * cols:(q + 1) * cols]
            flat = np.ascontiguousarray(
                O.reshape(128, Bc // 512, 4).transpose(1, 2, 0)).reshape(Bc)
            if im == 0:
                E[:, dst] = flat
            else:
                E[:, dst] += 1j * flat.astype(np.complex64)
        outs.append(E)
    return np.concatenate(outs, axis=0)


# revision 49
# speedup vs baseline: 1.0692x; 1.0692x over previous
"""Trainium2 Bass kernel for nn_EqAMPBC (FWM/XPM nonlinear equalizer), v2.

Data-parallel over 8 cores (batch 131072 -> 16384/core), 32 chunks of
N=512 samples in transposed layout (features on partitions, samples free).

v2 structure (vs v1):
  - Conjugate pairing: S[-m, m+n] = conj(S[m, n]) halves the triplet set
    to R=187 representatives; the conjugate contribution is folded into
    the w2 weight tables.
  - Gathers run as fp8e4 DoubleRow matmuls (half PE cost); the rhs is an
    interleaved [82, 2, N] (re, im) fp8 tile so one gather matmul can pull
    arbitrary mixes of real/imag rows.
  - Products are free-dim packed: one DVE op computes (ar*cr | ai*ci).
  - S crosses PSUM->SBUF via DMA (f32) and w2 runs with float32r moving
    operands (1 cycle/row), freeing Act/Pool for the gather copies.
  - Phase/center path (q, squares, xrA/B, r3, final) keeps v1 semantics.
"""
import sys
import numpy as np

sys.path.insert(0, "/opt/trn_rl_repo")

M = 41
P = 20
RHO = 1.0
NCORES = 8
N = 512
F_ROWS = 8
SLAB_REPS = 64  # reps per slab (2 product rows per rep)
R_KEEP = 187    # kept representatives; 187 = exact (lossy drop is too lossy)


def _fwm_index():
    h = M // 2
    ms, ns = [], []
    for m in range(-h, h + 1):
        for n in range(-h, h + 1):
            if m != 0 and n != 0 and abs(m * n) <= RHO * h and abs(m + n) <= h:
                ms.append(m)
                ns.append(n)
    return np.asarray(ms, np.int32), np.asarray(ns, np.int32)


M_IDX, N_IDX = _fwm_index()
HDIM = len(M_IDX)
H_OF = {(int(M_IDX[i]), int(N_IDX[i])): i for i in range(HDIM)}


def _build_reps():
    """Representatives under (m, n) -> (-m, m+n) conjugate pairing."""
    reps = []
    seen = set()
    for i in range(HDIM):
        m, n = int(M_IDX[i]), int(N_IDX[i])
        if (m, n) in seen:
            continue
        pm, pn = -m, m + n
        if (pm, pn) in H_OF and (pm, pn) != (m, n):
            reps.append((m, n, pm, pn))
            seen.add((m, n))
            seen.add((pm, pn))
        else:
            reps.append((m, n, None, None))
            seen.add((m, n))
    return reps


ALL_REPS = _build_reps()  # 187 representatives
R = min(R_KEEP, len(ALL_REPS))
SLABS = [min(SLAB_REPS, R - o) for o in range(0, R, SLAB_REPS)]
NSLAB = len(SLABS)
SLAB_OFF = [sum(SLABS[:i]) for i in range(NSLAB)]


def _select_reps(fwm_wr, fwm_wi):
    """Top-R representatives by conjugate-pair weight mass."""
    mass = []
    for (m, n, pm, pn) in ALL_REPS:
        h = H_OF[(m, n)]
        v = float(np.sum(fwm_wr[:, h] ** 2 + fwm_wi[:, h] ** 2))
        if pm is not None:
            hp = H_OF[(pm, pn)]
            v += float(np.sum(fwm_wr[:, hp] ** 2 + fwm_wi[:, hp] ** 2))
        mass.append(v)
    order = np.argsort(mass)[::-1][:R]
    return [ALL_REPS[i] for i in sorted(order)]


def _build_tables(fwm_wr, fwm_wi, conv1_w, conv2_w, C00):
    """Build packed constant tables.

    Returns dict:
      CT8  [128, sum(2*rows_s)*4] fp8-valued f32 (caller converts): DR gather
           one-hots, order: slab-major, group (ar, ai, cr, ci), layout per
           block = (comp-major, rows) flattened.
      CTB  [128, ...] bf16: w1 tables (re, imp, imm per slab) then r3 tables.
      CTF  [128, ...] f32: w2 (re, im per slab).
      plus r3 offsets metadata handled by fixed layout below.
    """
    t = {}
    reps = _select_reps(fwm_wr, fwm_wi)
    # --- gather tables (bf16, one-hot; rhs is x[:, comp, :]) ---
    blocks8 = []
    for s in range(NSLAB):
        rows = 2 * SLABS[s]
        for g in range(4):  # ar, ai, cr, ci
            tab = np.zeros((128, rows), np.float32)
            for rl in range(SLABS[s]):
                m, n, _, _ = reps[SLAB_OFF[s] + rl]
                tap = P + n if g < 2 else P + m + n
                for j in range(2):
                    tab[j * 41 + tap, 2 * rl + j] = 1.0
            blocks8.append(tab)
    t["CT8"] = np.concatenate(blocks8, axis=1)

    # --- w1 tables (bf16, +-1) ---
    blocksb = []
    for s in range(NSLAB):
        rows = 2 * SLABS[s]
        t_re = np.zeros((128, rows), np.float32)
        t_ip = np.zeros((128, rows), np.float32)
        t_im = np.zeros((128, rows), np.float32)
        for rl in range(SLABS[s]):
            for j in range(2):
                t_re[2 * rl + j, 2 * rl] = 1.0
                t_ip[2 * rl + j, 2 * rl + 1] = 1.0
                t_im[2 * rl + j, 2 * rl + 1] = -1.0
        blocksb += [t_re, t_ip, t_im]
    # --- r3 tables (bf16) ---
    w1z = conv1_w.copy(); w1z[P] = 0.0
    w2z = conv2_w.copy(); w2z[P] = 0.0
    q1 = np.zeros((128, F_ROWS), np.float32)
    q2 = np.zeros((128, F_ROWS), np.float32)
    q3 = np.zeros((128, F_ROWS), np.float32)
    q4 = np.zeros((128, F_ROWS), np.float32)
    pw = np.zeros((128, F_ROWS), np.float32)
    for i in range(2):
        rows = np.arange(41) + i * 41
        q1[rows, 2 * i] = 0.5
        q2[rows, 2 * i] = -0.5
        q3[rows, 2 * i + 1] = 0.5
        q4[rows, 2 * i + 1] = 0.5
        for tap in range(41):
            r = i * 41 + tap
            pw[r, 6] += (2.0 if i == 0 else 1.0) * w1z[tap]
            pw[r, 7] += (2.0 if i == 1 else 1.0) * w1z[tap]
        pw[i * 41 + P, 6] += 0.5 * C00
        pw[i * 41 + P, 7] += 0.5 * C00
    xrA = np.zeros((128, F_ROWS), np.float32)
    xrA[np.arange(41), 4] = 0.5 * w2z
    xrA[np.arange(41) + 64, 4] = 0.5 * w2z
    xrB = np.zeros((128, F_ROWS), np.float32)
    xrB[np.arange(41), 5] = 0.5 * w2z
    xrB[np.arange(41) + 64, 5] = -0.5 * w2z
    blocksb += [q1, q2, q3, q4, pw, xrA, xrB]
    t["CTB"] = np.concatenate(blocksb, axis=1)

    # --- w2 tables (f32, conj fold) ---
    blocksf = []
    for s in range(NSLAB):
        rows = 2 * SLABS[s]
        wre = np.zeros((128, 82), np.float32)
        wim = np.zeros((128, 82), np.float32)
        for rl in range(SLABS[s]):
            m, n, pm, pn = reps[SLAB_OFF[s] + rl]
            h = H_OF[(m, n)]
            for i in range(2):
                col = i * 41 + P + m
                wre[2 * rl + 0, col] += fwm_wr[i, h]
                wre[2 * rl + 1, col] += -fwm_wi[i, h]
                wim[2 * rl + 0, col] += fwm_wi[i, h]
                wim[2 * rl + 1, col] += fwm_wr[i, h]
            if pm is not None:
                hp = H_OF[(pm, pn)]
                for i in range(2):
                    col = i * 41 + P + pm
                    wre[2 * rl + 0, col] += fwm_wr[i, hp]
                    wre[2 * rl + 1, col] += fwm_wi[i, hp]
                    wim[2 * rl + 0, col] += fwm_wi[i, hp]
                    wim[2 * rl + 1, col] += -fwm_wr[i, hp]
        blocksf += [wre, wim]
    t["CTF"] = np.concatenate(blocksf, axis=1)  # bf16 on device
    t["ident8"] = np.eye(F_ROWS, dtype=np.float32)
    return t


_CACHED = {}


def _build_program(Bc):
    import concourse.bacc as bacc
    import concourse.mybir as mybir
    import concourse.tile as tile

    f32 = mybir.dt.float32
    bf16 = mybir.dt.bfloat16
    Act = mybir.ActivationFunctionType
    Op = mybir.AluOpType
    NCHUNK = Bc // N
    MCOLS = Bc // 128

    # const layout offsets
    ct8_off, o = [], 0
    for s in range(NSLAB):
        for g in range(4):
            ct8_off.append(o)
            o += 2 * SLABS[s]
    CT8_COLS = o
    ctb_off, o = {}, 0
    for s in range(NSLAB):
        for nm in ("re", "ip", "im"):
            ctb_off[(s, nm)] = o
            o += 2 * SLABS[s]
    for nm in ("q1", "q2", "q3", "q4", "pw", "xrA", "xrB"):
        ctb_off[nm] = o
        o += F_ROWS
    CTB_COLS = o
    ctf_off, o = {}, 0
    for s in range(NSLAB):
        for nm in ("re", "im"):
            ctf_off[(s, nm)] = o
            o += 82
    CTF_COLS = o

    nc = bacc.Bacc("TRN2", target_bir_lowering=False, debug=False,
                   num_devices=NCORES)

    dXP = nc.dram_tensor("XP", [82, 2, Bc], bf16, kind="ExternalInput").ap()
    dXP2 = nc.dram_tensor("XP2", [82, 2, Bc], bf16, kind="ExternalInput").ap()
    dXC = nc.dram_tensor("XC", [128, 4 * MCOLS], f32, kind="ExternalInput").ap()
    dT0 = nc.dram_tensor("T0M", [128, MCOLS], f32, kind="ExternalInput").ap()
    dCT8 = nc.dram_tensor("CT8", [128, CT8_COLS], bf16, kind="ExternalInput").ap()
    dCTB = nc.dram_tensor("CTB", [128, CTB_COLS], bf16, kind="ExternalInput").ap()
    dCTF = nc.dram_tensor("CTF", [128, CTF_COLS], bf16, kind="ExternalInput").ap()
    dID8 = nc.dram_tensor("ID8F", [F_ROWS, F_ROWS], f32,
                          kind="ExternalInput").ap()
    dOUT = nc.dram_tensor("OUT", [128, 4 * MCOLS], f32,
                          kind="ExternalOutput").ap()

    with tile.TileContext(nc) as tc:
        with (
            tc.tile_pool(name="consts", bufs=1) as cpool,
            tc.tile_pool(name="xin", bufs=3) as xpool,
            tc.tile_pool(name="gsb", bufs=3) as gpool,
            tc.tile_pool(name="prod", bufs=2) as ppool,
            tc.tile_pool(name="ssb", bufs=2) as spool,
            tc.tile_pool(name="mid", bufs=2) as mpool,
            tc.tile_pool(name="persist", bufs=1) as zpool,
            tc.tile_pool(name="fin", bufs=4) as fpool,
            tc.tile_pool(name="pga", bufs=1, space="PSUM") as pga,
            tc.tile_pool(name="pgc", bufs=1, space="PSUM") as pgc,
            tc.tile_pool(name="pss", bufs=1, space="PSUM") as pss,
            tc.tile_pool(name="pvv", bufs=1, space="PSUM") as pvv,
            tc.tile_pool(name="pff", bufs=1, space="PSUM") as pff,
            tc.tile_pool(name="pgg", bufs=1, space="PSUM") as pgg,
        ):
            # ---- constants (gather tables first; x DMAs slot in between) ----
            ct8 = cpool.tile([128, CT8_COLS], bf16, tag="ct8", name="ct8")
            ctb = cpool.tile([128, CTB_COLS], bf16, tag="ctb", name="ctb")
            ctf = cpool.tile([128, CTF_COLS], bf16, tag="ctf", name="ctf")
            t0m = cpool.tile([128, MCOLS], f32, tag="t0m", name="t0m")
            xcs = cpool.tile([128, 4 * MCOLS], f32, tag="xcs", name="xcs")
            ident8f = cpool.tile([F_ROWS, F_ROWS], f32, tag="id8",
                                 name="ident8f")
            CT8_S0 = ct8_off[4]  # end of slab-0 tables
            nc.sync.dma_start(ct8[:, 0:CT8_S0], dCT8[:, 0:CT8_S0])

            def gq(s, g):
                rows = 2 * SLABS[s]
                off = ct8_off[4 * s + g]
                return ct8[0:82, off:off + rows]

            def w1t(s, nm):
                rows = 2 * SLABS[s]
                off = ctb_off[(s, nm)]
                return ctb[0:rows, off:off + rows]

            def w2t(s, nm):
                rows = 2 * SLABS[s]
                off = ctf_off[(s, nm)]
                return ctf[0:rows, off:off + 82]

            def r3t(nm, np_=128):
                off = ctb_off[nm]
                return ctb[0:np_, off:off + F_ROWS]

            Mt = zpool.tile([128, NCHUNK * 32], f32, tag="mega", name="mega")
            xrAB = []
            for par in range(2):
                a = zpool.tile([128, N], bf16, tag=f"xrA{par}", name=f"xrA{par}")
                b = zpool.tile([128, N], bf16, tag=f"xrB{par}", name=f"xrB{par}")
                nc.vector.memset(a[:], 0.0)
                nc.vector.memset(b[:], 0.0)
                xrAB.append((a, b))
            OUTs = zpool.tile([128, 4 * MCOLS], f32, tag="outs", name="outs")

            # ---- software-pipelined chunk loop ----
            def new_state(c):
                st = {"c": c}
                st["x"] = xpool.tile([82, 2, N], bf16, tag="x", name="x")
                st["x2"] = xpool.tile([82, 2, N], bf16, tag="x2", name="x2")
                st["xm1"] = xpool.tile([41, 2, N], bf16, tag="xm1", name="xm1")
                st["xrA"], st["xrB"] = xrAB[c % 2]
                return st

            def emit_dmas(st):
                cs = slice(st["c"] * N, (st["c"] + 1) * N)
                nc.sync.dma_start(st["x"][:], dXP[:, :, cs])
                nc.sync.dma_start(st["x2"][:], dXP2[:, :, cs])
                nc.sync.dma_start(st["xm1"][:], dXP[41:82, :, cs])

            def emit_gather_a(st, s):
                rows = 2 * SLABS[s]
                pGA = pga.tile([128, 2, N], f32, tag="pga", name="pGA")
                for g in (0, 1):
                    nc.tensor.matmul(pGA[0:rows, g, :], gq(s, g),
                                     st["x"][:, g % 2, :],
                                     start=True, stop=True)
                st[f"pGA{s}"] = pGA

            def emit_gather_c(st, s):
                rows = 2 * SLABS[s]
                pGC = pgc.tile([128, 2, N], f32, tag="pgc", name="pGC")
                for g in (2, 3):
                    nc.tensor.matmul(pGC[0:rows, g - 2, :], gq(s, g),
                                     st["x"][:, g % 2, :],
                                     start=True, stop=True)
                st[f"pGC{s}"] = pGC

            def emit_ga_copy(st, s):
                rows = 2 * SLABS[s]
                ga = gpool.tile([128, 2, N], bf16, tag="gas", name="ga")
                nc.scalar.activation(ga[0:rows, :, :],
                                     st[f"pGA{s}"][0:rows, :, :], Act.Copy)
                st[f"ga{s}"] = ga

            def emit_gc_copy(st, s):
                rows = 2 * SLABS[s]
                gc = gpool.tile([128, 2, N], bf16, tag="gcs", name="gc")
                nc.scalar.activation(gc[0:rows, :, :],
                                     st[f"pGC{s}"][0:rows, :, :], Act.Copy)
                st[f"gc{s}"] = gc

            def emit_products(st, s, eng):
                rows = 2 * SLABS[s]
                ga, gc = st[f"ga{s}"], st[f"gc{s}"]
                p12 = ppool.tile([128, 2, N], bf16, tag="p12", name="p12")
                p3 = ppool.tile([128, N], bf16, tag="p3", name="p3")
                p4 = ppool.tile([128, N], bf16, tag="p4", name="p4")
                nc.vector.tensor_tensor(p12[0:rows, :, :], ga[0:rows, :, :],
                                        gc[0:rows, :, :], Op.mult)
                nc_e = nc.gpsimd if eng == "pool" else nc.vector
                nc_e.tensor_tensor(p3[0:rows, :], ga[0:rows, 1, :],
                                   gc[0:rows, 0, :], Op.mult)
                nc_e.tensor_tensor(p4[0:rows, :], ga[0:rows, 0, :],
                                   gc[0:rows, 1, :], Op.mult)
                st[f"p12{s}"], st[f"p3{s}"], st[f"p4{s}"] = p12, p3, p4

            def emit_w1(st, s):
                rows = 2 * SLABS[s]
                pS = pss.tile([128, N], f32, tag="s", name="pS")
                nc.tensor.matmul(pS[0:rows, :], w1t(s, "re"),
                                 st[f"p12{s}"][0:rows, 0, :],
                                 start=True, stop=False)
                nc.tensor.matmul(pS[0:rows, :], w1t(s, "re"),
                                 st[f"p12{s}"][0:rows, 1, :],
                                 start=False, stop=False)
                nc.tensor.matmul(pS[0:rows, :], w1t(s, "ip"),
                                 st[f"p3{s}"][0:rows, :],
                                 start=False, stop=False)
                nc.tensor.matmul(pS[0:rows, :], w1t(s, "im"),
                                 st[f"p4{s}"][0:rows, :],
                                 start=False, stop=True)
                st[f"pS{s}"] = pS

            def emit_scopy(st, s):
                rows = 2 * SLABS[s]
                ss = spool.tile([128, N], bf16, tag=f"ss{s}", name=f"ss{s}")
                nc.vector.tensor_copy(ss[0:rows, :], st[f"pS{s}"][0:rows, :])
                st[f"ss{s}"] = ss

            def emit_w2_mm(st, nm):
                if "pV" not in st:
                    st["pV"] = pvv.tile([82, N], f32, tag="v", name="pV")
                    st["VS"] = mpool.tile([82, 2, N], bf16, tag="vs", name="VS")
                for s in range(NSLAB):
                    rows = 2 * SLABS[s]
                    nc.tensor.matmul(st["pV"][:], w2t(s, nm),
                                     st[f"ss{s}"][0:rows, :],
                                     start=(s == 0), stop=(s == NSLAB - 1))

            def emit_v_copy(st, ci):
                if ci == 0:
                    nc.scalar.activation(st["VS"][:, 0, :], st["pV"][:],
                                         Act.Copy)
                else:
                    nc.vector.tensor_copy(st["VS"][:, 1, :], st["pV"][:])

            def emit_q(st):
                st["Q12"] = mpool.tile([82, 2, N], bf16, tag="q12", name="Q12")
                st["Q34"] = mpool.tile([82, 2, N], bf16, tag="q34", name="Q34")
                nc.vector.tensor_tensor(st["Q12"][:], st["VS"][:], st["x"][:],
                                        Op.mult)
                nc.vector.tensor_tensor(st["Q34"][:], st["VS"][:], st["x2"][:],
                                        Op.mult)

            def emit_s12_xr(st):
                x, xm1 = st["x"], st["xm1"]
                st["S12"] = mpool.tile([82, 2, N], bf16, tag="s12", name="S12")
                nc.vector.tensor_tensor(st["S12"][:], x[:], x[:], Op.mult)
                nc.vector.tensor_tensor(st["xrA"][0:41, :], x[0:41, 0, :],
                                        xm1[:, 0, :], Op.mult)
                nc.gpsimd.tensor_tensor(st["xrA"][64:105, :], x[0:41, 1, :],
                                        xm1[:, 1, :], Op.mult)
                nc.vector.tensor_tensor(st["xrB"][0:41, :], x[0:41, 1, :],
                                        xm1[:, 0, :], Op.mult)
                nc.gpsimd.tensor_tensor(st["xrB"][64:105, :], x[0:41, 0, :],
                                        xm1[:, 1, :], Op.mult)

            def emit_r3(st):
                pF = pff.tile([F_ROWS, N], f32, tag="f", name="pF")
                seq = [
                    (r3t("q1", 82), st["Q12"][:, 0, :]),
                    (r3t("q2", 82), st["Q12"][:, 1, :]),
                    (r3t("q3", 82), st["Q34"][:, 0, :]),
                    (r3t("q4", 82), st["Q34"][:, 1, :]),
                    (r3t("pw", 82), st["S12"][:, 0, :]),
                    (r3t("pw", 82), st["S12"][:, 1, :]),
                    (r3t("xrA", 128), st["xrA"][:]),
                    (r3t("xrB", 128), st["xrB"][:]),
                ]
                for si, (wt, rhs) in enumerate(seq):
                    nc.tensor.matmul(pF[:], wt, rhs,
                                     start=(si == 0), stop=(si == len(seq) - 1))
                st["pF"] = pF

            def emit_sf_tr_mt(st):
                c = st["c"]
                sF = mpool.tile([F_ROWS, N], f32, tag="sF", name="sF")
                nc.scalar.activation(sF[:], st["pF"][:], Act.Copy)
                pG = pgg.tile([128, 32], f32, tag="g", name="pG")
                for tq in range(4):
                    nc.tensor.transpose(pG[:, tq * 8:tq * 8 + 8],
                                        sF[:, tq * 128:(tq + 1) * 128],
                                        ident8f[:])
                nc.vector.tensor_copy(Mt[:, c * 32:(c + 1) * 32], pG[:])

            # ---- final sample-major phase, emitted in column halves ----
            Mtv = Mt[:].rearrange("p (g k) -> p g k", k=8)
            hpi = cpool.tile([128, 1], f32, tag="hpi", name="hpi")
            LN10_10 = float(np.log(10.0) / 10.0)

            FIN_COMBOS = [
                (0, [(2, 5, -1.0), (3, 4, -1.0)], (0, "c0", +1.0), (1, "s0", -1.0), 0),
                (1, [(2, 4, +1.0), (3, 5, -1.0)], (0, "s0", +1.0), (1, "c0", +1.0), 1),
                (2, [(0, 5, +1.0), (1, 4, -1.0)], (2, "c1", +1.0), (3, "s1", -1.0), 2),
                (3, [(0, 4, +1.0), (1, 5, +1.0)], (2, "s1", +1.0), (3, "c1", +1.0), 3),
            ]

            def emit_final_trig(fs, g0, g1, tg):
                gw = g1 - g0

                def ft(tag):
                    return fpool.tile([128, gw], f32, tag=tag + tg,
                                      name="ftmp")

                fs["Pht"] = ft("fA")
                nc.scalar.activation(fs["Pht"][:], t0m[:, g0:g1], Act.Exp,
                                     scale=LN10_10)
                phi0, phi1 = ft("fB0"), ft("fB1")
                ve = fs["ve"]
                ve.tensor_tensor(phi0[:], fs["Pht"][:], Mtv[:, g0:g1, 6],
                                 Op.mult)
                ve.tensor_tensor(phi1[:], fs["Pht"][:], Mtv[:, g0:g1, 7],
                                 Op.mult)
                tr = {}
                for nm, ph, bias in (("c0", phi0, True), ("s0", phi0, False),
                                     ("c1", phi1, True), ("s1", phi1, False)):
                    t = ft("fC" + nm)
                    if bias:
                        nc.scalar.activation(t[:], ph[:], Act.Sin, bias=hpi[:])
                    else:
                        nc.scalar.activation(t[:], ph[:], Act.Sin)
                    tr[nm] = t
                fs["trig"] = tr

            def emit_final_combo(fs, g0, g1, combo, tg):
                gw = g1 - g0
                fidx, prods, term1, term2, outq = combo
                ve = fs["ve"]

                def xcb(q):
                    return xcs[:, q * MCOLS + g0:q * MCOLS + g1]

                acc = fpool.tile([128, gw], f32, tag="fD" + tg, name="facc")
                ve.tensor_copy(acc[:], Mtv[:, g0:g1, fidx])
                for (ka, kb, sgn) in prods:
                    tmp = fpool.tile([128, gw], f32, tag="fE" + tg, name="ftp")
                    ve.tensor_tensor(tmp[:], xcb(ka), Mtv[:, g0:g1, kb],
                                     Op.mult)
                    ve.tensor_tensor(acc[:], acc[:], tmp[:],
                                     Op.add if sgn > 0 else Op.subtract)
                ve.tensor_tensor(acc[:], acc[:], fs["Pht"][:], Op.mult)
                for (kc, tkey, sgn) in (term1, term2):
                    tmp = fpool.tile([128, gw], f32, tag="fE" + tg, name="ftp")
                    ve.tensor_tensor(tmp[:], xcb(kc), fs["trig"][tkey][:],
                                     Op.mult)
                    ve.tensor_tensor(acc[:], acc[:], tmp[:],
                                     Op.add if sgn > 0 else Op.subtract)
                ve.tensor_copy(OUTs[:, outq * MCOLS + g0:outq * MCOLS + g1],
                               acc[:])

            # ---- pipelined main loop ----
            nxt = new_state(0)
            emit_dmas(nxt)
            nc.sync.dma_start(ct8[:, CT8_S0:], dCT8[:, CT8_S0:])
            nc.sync.dma_start(ctb[:], dCTB[:])
            nc.sync.dma_start(ctf[:], dCTF[:])
            nc.sync.dma_start(t0m[:], dT0[:])
            nc.sync.dma_start(xcs[:], dXC[:])
            nc.sync.dma_start(ident8f[:], dID8[:])
            nc.vector.memset(hpi[:], float(np.pi / 2))
            prv = None
            for i in range(NCHUNK + 1):
                cur = nxt if i < NCHUNK else None
                nxt = new_state(i + 1) if i + 1 < NCHUNK else None
                if nxt is not None:
                    emit_dmas(nxt)
                if cur is not None:
                    emit_gather_a(cur, 0)       # PE x2
                if prv is not None:
                    emit_w2_mm(prv, "re")       # PE x3
                    emit_v_copy(prv, 0)         # Act Vre
                if cur is not None:
                    emit_ga_copy(cur, 0)        # Act
                    emit_gather_c(cur, 0)       # PE x2
                if prv is not None:
                    emit_w2_mm(prv, "im")       # PE x3
                    emit_v_copy(prv, 1)         # DVE Vim
                if cur is not None:
                    emit_gc_copy(cur, 0)        # Act
                if prv is not None:
                    emit_q(prv)                 # DVE x2
                if cur is not None:
                    emit_gather_a(cur, 1)       # PE x2
                    emit_products(cur, 0, "pool")
                    emit_ga_copy(cur, 1)        # Act
                if prv is not None:
                    emit_r3(prv)                # PE x8
                if cur is not None:
                    emit_gather_c(cur, 1)       # PE x2
                    emit_gc_copy(cur, 1)        # Act
                    emit_w1(cur, 0)             # PE x4
                    emit_scopy(cur, 0)          # DVE
                    emit_gather_a(cur, 2)       # PE x2
                    emit_products(cur, 1, "pool")
                    emit_ga_copy(cur, 2)        # Act
                if prv is not None:
                    emit_sf_tr_mt(prv)          # Act sF + PE tr + DVE Mt
                if cur is not None:
                    emit_gather_c(cur, 2)       # PE x2
                    emit_gc_copy(cur, 2)        # Act
                    emit_w1(cur, 1)             # PE x4
                    emit_scopy(cur, 1)          # DVE
                    emit_products(cur, 2, "dve")
                    emit_s12_xr(cur)            # DVE + Pool
                    emit_w1(cur, 2)             # PE x4
                    emit_scopy(cur, 2)          # DVE
                prv = cur
            # final phase at the end, combos split across DVE and Pool
            finB = {"ve": nc.vector}
            finC = {"ve": nc.gpsimd}
            emit_final_trig(finB, 0, MCOLS, "B")
            finC["Pht"], finC["trig"] = finB["Pht"], finB["trig"]
            emit_final_combo(finB, 0, MCOLS, FIN_COMBOS[0], "B")
            emit_final_combo(finC, 0, MCOLS, FIN_COMBOS[1], "C")
            emit_final_combo(finB, 0, MCOLS, FIN_COMBOS[2], "B")
            emit_final_combo(finC, 0, MCOLS, FIN_COMBOS[3], "C")
            nc.sync.dma_start(dOUT[:], OUTs[:])

    nc.compile()
    return nc


def kernel(**inputs):
    from concourse.bass_utils import run_bass_kernel_spmd
    import ml_dtypes

    trace = bool(inputs.pop("_trace", False))
    x_real = np.asarray(inputs["x_real"], dtype=np.float32)
    x_imag = np.asarray(inputs["x_imag"], dtype=np.float32)
    task_info = np.asarray(inputs["task_info"], dtype=np.float32)
    C00 = float(np.asarray(inputs["C00"]).reshape(-1)[0])
    fwm_wr = np.asarray(inputs["fwm_wr"], dtype=np.float32)
    fwm_wi = np.asarray(inputs["fwm_wi"], dtype=np.float32)
    conv1_w = np.asarray(inputs["conv1_w"], dtype=np.float32)
    conv2_w = np.asarray(inputs["conv2_w"], dtype=np.float32)

    B = x_real.shape[0]
    Bc = B // NCORES
    if "nc" not in _CACHED:
        _CACHED["nc"] = _build_program(Bc)
    nc = _CACHED["nc"]

    bf = ml_dtypes.bfloat16
    tabs = _build_tables(fwm_wr, fwm_wi, conv1_w, conv2_w, C00)
    CT8 = tabs["CT8"].astype(bf)
    CTB = tabs["CTB"].astype(bf)
    CTF = tabs["CTF"].astype(bf)
    ID8 = tabs["ident8"]

    in_maps = []
    for core in range(NCORES):
        sl = slice(core * Bc, (core + 1) * Bc)
        XPr = np.ascontiguousarray(
            x_real[sl].transpose(2, 1, 0).reshape(82, Bc))
        XPi = np.ascontiguousarray(
            x_imag[sl].transpose(2, 1, 0).reshape(82, Bc))
        XP = np.ascontiguousarray(np.stack([XPr, XPi], axis=1)).astype(bf)
        XP2 = np.ascontiguousarray(np.stack([XPi, XPr], axis=1)).astype(bf)
        t0 = task_info[sl, 0]
        T0M = np.ascontiguousarray(
            t0.reshape(Bc // 512, 4, 128).transpose(2, 0, 1).reshape(128, Bc // 128))
        mcols = Bc // 128
        XC = np.empty((128, 4 * mcols), np.float32)
        for qi, arr in enumerate([x_real[sl, P, 0], x_imag[sl, P, 0],
                                  x_real[sl, P, 1], x_imag[sl, P, 1]]):
            XC[:, qi * mcols:(qi + 1) * mcols] = np.ascontiguousarray(
                arr.reshape(Bc // 512, 4, 128).transpose(2, 0, 1).reshape(128, mcols))
        m = {"XP": XP, "XP2": XP2, "T0M": T0M, "XC": XC,
             "CT8": CT8, "CTB": CTB, "CTF": CTF, "ID8F": ID8}
        in_maps.append(m)

    res = run_bass_kernel_spmd(nc, in_maps, list(range(NCORES)), trace=trace)
    _CACHED["last_exec_ns"] = res.exec_time_ns

    outs = []
    cols = Bc // 128
    for core in range(NCORES):
        OUT = res.results[core]["OUT"]
        E = np.empty((Bc, 2), np.complex64)
        for q, (dst, im) in enumerate([(0, 0), (0, 1), (1, 0), (1, 1)]):
            O = OUT[:, q * cols:(q + 1) * cols]
            flat = np.ascontiguousarray(
                O.reshape(128, Bc // 512, 4).transpose(1, 2, 0)).reshape(Bc)
            if im == 0:
                E[:, dst] = flat
            else:
                E[:, dst] += 1j * flat.astype(np.complex64)
        outs.append(E)
    return np.concatenate(outs, axis=0)
